# revision 1
# baseline (speedup 1.0000x reference)
"""Trainium2 Bass kernel for 2-layer GIN (DGI) message passing.

Reference computation (per layer, mean aggregation, eps=0):
    agg = segment_sum(h[src], dst) / max(deg,1)
    h'  = relu((h + agg) @ W.T + b)

Linearity trick: (h + agg(h)) @ W.T = y + agg(y) with y = h @ W.T, so both
layers aggregate 128-wide features:
    y1 = h @ W1.T          ; h1 = relu(y1 + agg(y1)*inv_deg + b1)
    y2 = h1 @ W2.T         ; h2 = relu(y2 + agg(y2)*inv_deg + b2)

Distribution: nodes sharded 12500/core across 8 cores (degree-balanced
permutation), per-layer AllGather of the projected features y (the gather
table), per-edge messages fetched with GPSIMD dma_gather (int16 indices ->
4 source groups of 25000 rows), segment-sum via one-hot selection-matrix
matmuls accumulated in PSUM (edges sorted by destination range).
"""

import math
import time
import numpy as np

import concourse.bass as bass
import concourse.bacc as bacc
import concourse.tile as tile
from concourse import bass2jax, mybir

P = 128
NCORES = 8
N = 100000
E = 3200000
IN_FEATS = 256
H_FEATS = 128
NB = N // NCORES             # 12500 nodes per core
NRANGES = math.ceil(NB / P)  # 98 (last range has 84 slots)
NGROUPS = 4
GROUP_ROWS = N // NGROUPS    # 25000 (< 32768, fits int16 index)

MSG_BF16 = True              # gather table + messages in bf16
W_BATCH = 6                  # ranges per gather instruction (SBUF budget)


def _bf16():
    import ml_dtypes
    return ml_dtypes.bfloat16


# ---------------------------------------------------------------- host side

def _snake_fill(items, nbins, caps):
    """Distribute items (in given order) over bins in snake order, skipping
    full bins. Returns list of lists."""
    buckets = [[] for _ in range(nbins)]
    caps = list(caps)
    b, d = 0, 1
    for it in items:
        while caps[b] == 0:
            nb = b + d
            if nb < 0 or nb >= nbins:
                d = -d
                nb = b + d
            b = nb
        buckets[b].append(it)
        caps[b] -= 1
        nb = b + d
        if nb < 0 or nb >= nbins:
            d = -d
        else:
            b = nb
    return buckets


def _balance_partition(deg):
    """old node id -> new node id; new layout: core*NB + within-core index,
    range r = within-core indices [r*128, min((r+1)*128, NB))."""
    order = np.argsort(-deg, kind="stable")
    cores = _snake_fill(order, NCORES, [NB] * NCORES)
    perm = np.empty(N, np.int64)
    for ci in range(NCORES):
        nodes = np.asarray(cores[ci])
        nodes = nodes[np.argsort(-deg[nodes], kind="stable")]
        caps = [min(P, NB - r * P) for r in range(NRANGES)]
        rbuckets = _snake_fill(nodes, NRANGES, caps)
        idx = ci * NB
        for r in range(NRANGES):
            for node in rbuckets[r]:
                perm[node] = idx
                idx += 1
        assert idx == (ci + 1) * NB
    return perm


def preprocess(h, W1, b1, W2, b2, src, dst):
    src = np.asarray(src).astype(np.int64)
    dst = np.asarray(dst).astype(np.int64)
    deg = np.bincount(dst, minlength=N).astype(np.int64)
    inv_deg = (1.0 / np.maximum(deg, 1.0)).astype(np.float32)

    perm = _balance_partition(deg)          # old -> new
    inv_perm = np.empty(N, np.int64)
    inv_perm[perm] = np.arange(N)

    src_n = perm[src]
    dst_n = perm[dst]

    core_e = dst_n // NB
    grp_e = src_n // GROUP_ROWS
    order = np.lexsort((dst_n, grp_e, core_e))
    src_n, dst_n, core_e, grp_e = (
        src_n[order], dst_n[order], core_e[order], grp_e[order])

    loc = dst_n - core_e * NB
    rng_e = loc // P
    cell_id = ((core_e * NRANGES) + rng_e) * NGROUPS + grp_e
    counts = np.bincount(cell_id, minlength=NCORES * NRANGES * NGROUPS)
    counts = counts.reshape(NCORES, NRANGES, NGROUPS)
    T = np.maximum(1, np.ceil(counts.max(axis=0) / P).astype(np.int64))

    CT = T.sum(axis=1)
    CTOFF = np.concatenate([[0], np.cumsum(CT)])
    TOT_G = T.sum(axis=0) * P
    batches = [list(range(b, min(b + W_BATCH, NRANGES)))
               for b in range(0, NRANGES, W_BATCH)]

    mnp = _bf16() if MSG_BF16 else np.float32
    iota = np.broadcast_to(np.arange(P, dtype=np.float32), (P, P)).copy()

    in_maps = []
    for c in range(NCORES):
        sel = core_e == c
        s_c, g_c = src_n[sel], grp_e[sel]
        loc_c = dst_n[sel] - c * NB
        r_c = loc_c // P
        slot_c = loc_c - r_c * P

        idx_g = [np.zeros(TOT_G[g], np.int16) for g in range(NGROUPS)]
        dstp = np.full((P, int(CT.sum())), 255, np.float32)
        for g in range(NGROUPS):
            gsel = g_c == g
            sg, rg, slg = s_c[gsel], r_c[gsel], slot_c[gsel]
            rng_counts = np.bincount(rg, minlength=NRANGES)
            off = 0
            pos = 0
            for r in range(NRANGES):
                cnt = int(rng_counts[r])
                L = int(T[r, g]) * P
                assert cnt <= L, (c, r, g, cnt, L)
                idx_g[g][pos:pos + cnt] = (sg[off:off + cnt]
                                           - g * GROUP_ROWS).astype(np.int16)
                colbase = int(CTOFF[r]) + int(T[r, :g].sum())
                flat = np.full(L, 255, np.float32)
                flat[:cnt] = slg[off:off + cnt]
                dstp[:, colbase:colbase + int(T[r, g])] = (
                    flat.reshape(int(T[r, g]), P).T)
                off += cnt
                pos += L
            assert pos == TOT_G[g]

        idx_wrapped = []
        for g in range(NGROUPS):
            wr = idx_g[g].reshape(-1, 16).T
            idx_wrapped.append(np.tile(wr, (8, 1)).copy())

        # one-hot selection matrices, fp8 (exact 0/1), [P, CTsum*128]
        import ml_dtypes
        S_all = (dstp[:, :, None] == np.arange(P, dtype=np.float32)[None, None, :]
                 ).astype(ml_dtypes.float8_e4m3fn).reshape(P, int(CT.sum()) * P)

        own_old = inv_perm[np.arange(c * NB, (c + 1) * NB)]
        hT = np.ascontiguousarray(h[own_old].T).astype(np.float32)
        ivd = np.ones(NRANGES * P, np.float32)
        ivd[:NB] = inv_deg[own_old]
        invdegT = np.ascontiguousarray(ivd.reshape(NRANGES, P).T)

        m = {
            "hT": hT,
            "invdegT": invdegT,
            "S_all": S_all,
            "W1T": np.ascontiguousarray(W1.T).astype(np.float32),
            "W2T": np.ascontiguousarray(W2.T).astype(np.float32),
            "b1_rep": np.broadcast_to(b1, (P, H_FEATS)).copy().astype(np.float32),
            "b2_rep": np.broadcast_to(b2, (P, H_FEATS)).copy().astype(np.float32),
            "identity": np.eye(P, dtype=np.float32),
        }
        for g in range(NGROUPS):
            m[f"idx{g}"] = idx_wrapped[g]
        in_maps.append(m)

    meta = dict(T=T, CT=CT, CTOFF=CTOFF, TOT_G=TOT_G, batches=batches,
                perm=perm, inv_perm=inv_perm)
    return in_maps, meta


# ------------------------------------------------------------- device build

def build_program(meta, no_collectives=False, skip=()):
    T, CT, CTOFF, batches = meta["T"], meta["CT"], meta["CTOFF"], meta["batches"]
    TOT_G = meta["TOT_G"]

    nc = bacc.Bacc("TRN2", target_bir_lowering=False, debug=False,
                   num_devices=NCORES, num_swdge_queues=4)
    f32 = mybir.dt.float32
    i16 = mybir.dt.int16
    mdt = mybir.dt.bfloat16 if MSG_BF16 else f32

    hT_d = nc.dram_tensor("hT", [IN_FEATS, NB], f32, kind="ExternalInput")
    invdegT_d = nc.dram_tensor("invdegT", [P, NRANGES], f32, kind="ExternalInput")
    S_d = nc.dram_tensor("S_all", [P, int(CT.sum()) * P], mybir.dt.float8e4,
                         kind="ExternalInput")
    W1T_d = nc.dram_tensor("W1T", [IN_FEATS, H_FEATS], f32, kind="ExternalInput")
    W2T_d = nc.dram_tensor("W2T", [H_FEATS, H_FEATS], f32, kind="ExternalInput")
    b1_d = nc.dram_tensor("b1_rep", [P, H_FEATS], f32, kind="ExternalInput")
    b2_d = nc.dram_tensor("b2_rep", [P, H_FEATS], f32, kind="ExternalInput")
    ident_d = nc.dram_tensor("identity", [P, P], f32, kind="ExternalInput")
    idx_d = [nc.dram_tensor(f"idx{g}", [P, int(TOT_G[g]) // 16], i16,
                            kind="ExternalInput") for g in range(NGROUPS)]
    out_d = nc.dram_tensor("out", [NB, H_FEATS], f32, kind="ExternalOutput")

    with tile.TileContext(nc) as tc:
        with (
            tc.tile_pool(name="const", bufs=1) as cpool,
            tc.tile_pool(name="sb", bufs=2) as sb,
            tc.tile_pool(name="slab", bufs=2) as slabp,
            tc.tile_pool(name="ps_agg", bufs=2, space="PSUM") as ps_agg,
            tc.tile_pool(name="ps_tr", bufs=2, space="PSUM") as ps_tr,
            tc.tile_pool(name="ps_y2", bufs=2, space="PSUM") as ps_y2,
            tc.tile_pool(name="ps_a", bufs=2, space="PSUM") as ps_a,
            tc.tile_pool(name="dram", bufs=1, space="DRAM") as dram,
        ):
            y1_own = dram.tile([NB, H_FEATS], f32, tag="y1own")
            y2_own = dram.tile([NB, H_FEATS], f32, tag="y2own")
            y1_bounce = dram.tile([NB, H_FEATS], mdt, tag="y1b")
            y2_bounce = dram.tile([NB, H_FEATS], mdt, tag="y2b")
            y1_full = dram.tile([N, H_FEATS], mdt, tag="y1f")
            y2_full = dram.tile([N, H_FEATS], mdt, tag="y2f")

            invdegT_t = cpool.tile([P, NRANGES], f32, tag="ivd")
            nc.sync.dma_start(out=invdegT_t[:], in_=invdegT_d[:])
            W1T_t = [cpool.tile([P, H_FEATS], f32, tag=f"w1_{ch}", name=f"w1_{ch}")
                     for ch in range(IN_FEATS // P)]
            for ch in range(IN_FEATS // P):
                nc.sync.dma_start(out=W1T_t[ch][:],
                                  in_=W1T_d[ch * P:(ch + 1) * P, :])
            W2T_t = cpool.tile([P, H_FEATS], f32, tag="w2")
            nc.sync.dma_start(out=W2T_t[:], in_=W2T_d[:])
            b1_t = cpool.tile([P, H_FEATS], f32, tag="b1")
            nc.sync.dma_start(out=b1_t[:], in_=b1_d[:])
            b2_t = cpool.tile([P, H_FEATS], f32, tag="b2")
            nc.sync.dma_start(out=b2_t[:], in_=b2_d[:])
            ident_t = cpool.tile([P, P], f32, tag="ident")
            nc.sync.dma_start(out=ident_t[:], in_=ident_d[:])

            # ---------------- phase A: y1 = hT.T @ W1T (own block)
            for r in range(NRANGES):
                rows = min(P, NB - r * P)
                hT_t = [sb.tile([P, P], f32, tag=f"hT{ch}", name=f"hT{ch}")
                        for ch in range(IN_FEATS // P)]
                for ch in range(IN_FEATS // P):
                    nc.sync.dma_start(
                        out=hT_t[ch][:, :rows],
                        in_=hT_d[ch * P:(ch + 1) * P, r * P:r * P + rows])
                y1_ps = ps_a.tile([P, H_FEATS], f32, tag="y1ps")
                for ch in range(IN_FEATS // P):
                    nc.tensor.matmul(
                        out=y1_ps[:], lhsT=hT_t[ch][:], rhs=W1T_t[ch][:],
                        start=(ch == 0), stop=(ch == IN_FEATS // P - 1))
                y1_sb = sb.tile([P, H_FEATS], f32, tag="y1sb")
                nc.scalar.activation(out=y1_sb[:], in_=y1_ps[:],
                                     func=mybir.ActivationFunctionType.Copy)
                nc.sync.dma_start(out=y1_own[r * P:r * P + rows, :],
                                  in_=y1_sb[:rows, :])
                if MSG_BF16:
                    y1_sbh = sb.tile([P, H_FEATS], mdt, tag="y1sbh")
                    nc.vector.tensor_copy(out=y1_sbh[:], in_=y1_ps[:])
                    nc.sync.dma_start(out=y1_bounce[r * P:r * P + rows, :],
                                      in_=y1_sbh[:rows, :])
                else:
                    nc.sync.dma_start(out=y1_bounce[r * P:r * P + rows, :],
                                      in_=y1_sb[:rows, :])

            if no_collectives:
                nc.sync.dma_start(out=y1_full[0:NB, :], in_=y1_bounce[:])
            else:
                nc.gpsimd.collective_compute(
                    "AllGather", mybir.AluOpType.bypass,
                    replica_groups=[list(range(NCORES))],
                    ins=[y1_bounce[:].opt()], outs=[y1_full[:].opt()])

            def layer(y_full_t, y_own_t, b_t, h_out_cb):
                goff = [0] * NGROUPS
                for batch in batches:
                    slabs = []
                    for g in range(NGROUPS):
                        ntiles = int(sum(T[r, g] for r in batch))
                        nidx = ntiles * P
                        idx_t = sb.tile([P, nidx // 16], i16, tag=f"idx{g}")
                        nc.sync.dma_start(
                            out=idx_t[:],
                            in_=idx_d[g][:, goff[g] * 8:(goff[g] + ntiles) * 8])
                        slab = slabp.tile([P, ntiles, H_FEATS], mdt,
                                          tag=f"slab{g}")
                        if "gather" in skip:
                            nc.vector.memset(slab[:, 0:1, :], 0)
                        else:
                            nc.gpsimd.dma_gather(
                                out_ap=slab[:],
                                in_ap=y_full_t[g * GROUP_ROWS:(g + 1) * GROUP_ROWS, :],
                                idxs_ap=idx_t[:], num_idxs=nidx, num_idxs_reg=nidx,
                                elem_size=H_FEATS, single_packet=False,
                                queue_num=g)
                        slabs.append(slab)

                    tilebase = [0] * NGROUPS
                    for r in batch:
                        ct = int(CT[r])
                        rows = min(P, NB - r * P)
                        S = sb.tile([P, ct, P], mybir.dt.float8e4, tag="S")
                        if "sload" not in skip:
                            nc.sync.dma_start(
                                out=S[:],
                                in_=S_d[:, int(CTOFF[r]) * P:(int(CTOFF[r]) + ct) * P
                                        ].rearrange("p (t s) -> p t s", s=P))

                        agg_ps = ps_agg.tile([P, H_FEATS], f32, tag="aggps")
                        if "matmul" in skip:
                            nc.tensor.matmul(
                                out=agg_ps[:], lhsT=S[:, 0, :],
                                rhs=S[:, 0, :],
                                start=True, stop=True)
                            for g in range(NGROUPS):
                                tilebase[g] += int(T[r, g])
                        else:
                            ncells = 0
                            for g in range(NGROUPS):
                                tg = int(T[r, g])
                                colbase = int(T[r, :g].sum())
                                for j in range(tg):
                                    nc.tensor.matmul(
                                        out=agg_ps[:],
                                        lhsT=S[:, colbase + j, :],
                                        rhs=slabs[g][:, tilebase[g] + j, :],
                                        start=(ncells == 0),
                                        stop=(ncells == int(CT[r]) - 1))
                                    ncells += 1
                                tilebase[g] += tg

                        yown_t = sb.tile([P, H_FEATS], f32, tag="yown")
                        nc.sync.dma_start(out=yown_t[:rows, :],
                                          in_=y_own_t[r * P:r * P + rows, :])
                        z = sb.tile([P, H_FEATS], f32, tag="z")
                        nc.vector.scalar_tensor_tensor(
                            out=z[:], in0=agg_ps[:],
                            scalar=invdegT_t[:, r:r + 1], in1=yown_t[:],
                            op0=mybir.AluOpType.mult, op1=mybir.AluOpType.add)
                        nc.vector.tensor_tensor(out=z[:], in0=z[:], in1=b_t[:],
                                                op=mybir.AluOpType.add)
                        h_t = sb.tile([P, H_FEATS], f32, tag="h")
                        nc.scalar.activation(
                            out=h_t[:], in_=z[:],
                            func=mybir.ActivationFunctionType.Relu)
                        h_out_cb(r, rows, h_t)
                    for g in range(NGROUPS):
                        goff[g] += int(sum(T[r, g] for r in batch))

            def l1_out(r, rows, h_t):
                h1T_ps = ps_tr.tile([P, P], f32, tag="h1Tps")
                nc.tensor.transpose(out=h1T_ps[:], in_=h_t[:],
                                    identity=ident_t[:])
                h1T_sb = sb.tile([P, P], f32, tag="h1Tsb")
                nc.scalar.activation(out=h1T_sb[:], in_=h1T_ps[:],
                                     func=mybir.ActivationFunctionType.Copy)
                y2_ps = ps_y2.tile([P, H_FEATS], f32, tag="y2ps")
                nc.tensor.matmul(out=y2_ps[:], lhsT=h1T_sb[:], rhs=W2T_t[:],
                                 start=True, stop=True)
                y2_sb = sb.tile([P, H_FEATS], f32, tag="y2sb")
                nc.scalar.activation(out=y2_sb[:], in_=y2_ps[:],
                                     func=mybir.ActivationFunctionType.Copy)
                nc.sync.dma_start(out=y2_own[r * P:r * P + rows, :],
                                  in_=y2_sb[:rows, :])
                if MSG_BF16:
                    y2_sbh = sb.tile([P, H_FEATS], mdt, tag="y2sbh")
                    nc.vector.tensor_copy(out=y2_sbh[:], in_=y2_ps[:])
                    nc.sync.dma_start(out=y2_bounce[r * P:r * P + rows, :],
                                      in_=y2_sbh[:rows, :])
                else:
                    nc.sync.dma_start(out=y2_bounce[r * P:r * P + rows, :],
                                      in_=y2_sb[:rows, :])

            layer(y1_full, y1_own, b1_t, l1_out)

            if no_collectives:
                nc.sync.dma_start(out=y2_full[0:NB, :], in_=y2_bounce[:])
            else:
                nc.gpsimd.collective_compute(
                    "AllGather", mybir.AluOpType.bypass,
                    replica_groups=[list(range(NCORES))],
                    ins=[y2_bounce[:].opt()], outs=[y2_full[:].opt()])

            def l2_out(r, rows, h_t):
                nc.sync.dma_start(out=out_d[r * P:r * P + rows, :],
                                  in_=h_t[:rows, :])

            layer(y2_full, y2_own, b2_t, l2_out)

    nc.compile()
    return nc


# ----------------------------------------------------------------- runner

def make_runner(nc, in_maps):
    """Reusable sharded executable over 8 cores (mirrors
    bass2jax.run_bass_via_pjrt but keeps the jitted fn + staged inputs).
    Returns (run, time_once) where run() -> list[dict] of outputs and
    time_once() -> wall seconds for one steady-state execution."""
    import jax
    from jax.sharding import Mesh, PartitionSpec
    from jax.experimental.shard_map import shard_map
    import concourse.mybir as mb

    bass2jax.install_neuronx_cc_hook()
    n_cores = len(in_maps)

    partition_name = (nc.partition_id_tensor.name
                      if nc.partition_id_tensor else None)
    in_names, out_names, out_avals, zero_outs = [], [], [], []
    for alloc in nc.m.functions[0].allocations:
        if not isinstance(alloc, mb.MemoryLocationSet):
            continue
        name = alloc.memorylocations[0].name
        if alloc.kind == "ExternalInput":
            if name != partition_name:
                in_names.append(name)
        elif alloc.kind == "ExternalOutput":
            out_names.append(name)
            shape = tuple(alloc.tensor_shape)
            dtype = mb.dt.np(alloc.dtype)
            out_avals.append(jax.core.ShapedArray(shape, dtype))
            zero_outs.append(np.zeros(shape, dtype))
    n_params = len(in_names)
    n_outs = len(out_avals)
    in_names_full = list(in_names) + out_names
    if partition_name is not None:
        in_names_full.append(partition_name)

    def _body(*args):
        operands = list(args)
        if partition_name is not None:
            operands.append(bass2jax.partition_id_tensor())
        outs = bass2jax._bass_exec_p.bind(
            *operands,
            out_avals=tuple(out_avals),
            in_names=tuple(in_names_full),
            out_names=tuple(out_names),
            lowering_input_output_aliases=(),
            sim_require_finite=True,
            sim_require_nnan=True,
            nc=nc,
        )
        return tuple(outs)

    devices = jax.devices()[:n_cores]
    mesh = Mesh(np.asarray(devices), ("core",))
    in_specs = (PartitionSpec("core"),) * (n_params + n_outs)
    out_specs = (PartitionSpec("core"),) * n_outs
    donate = tuple(range(n_params, n_params + n_outs))
    sharded = jax.jit(
        shard_map(_body, mesh=mesh, in_specs=in_specs, out_specs=out_specs,
                  check_rep=False),
        donate_argnums=donate, keep_unused=True)

    concat_in = [
        np.concatenate([np.asarray(in_maps[c][nm]) for c in range(n_cores)], 0)
        for nm in in_names]
    sharding = jax.sharding.NamedSharding(mesh, PartitionSpec("core"))
    staged = [jax.device_put(a, sharding) for a in concat_in]

    def _zeros():
        return [jax.device_put(
            np.zeros((n_cores * z.shape[0], *z.shape[1:]), z.dtype), sharding)
            for z in zero_outs]

    def run():
        out_arrs = sharded(*staged, *_zeros())
        jax.block_until_ready(out_arrs)
        return [
            {nm: np.asarray(out_arrs[i]).reshape(n_cores, *out_avals[i].shape)[c]
             for i, nm in enumerate(out_names)}
            for c in range(n_cores)]

    def time_once():
        zs = _zeros()
        jax.block_until_ready(zs)
        t0 = time.perf_counter()
        out_arrs = sharded(*staged, *zs)
        jax.block_until_ready(out_arrs)
        return time.perf_counter() - t0

    def time_slope(k=16):
        """Marginal device time per execution: queue k+1 executions without
        intermediate sync; slope vs a single execution."""
        zsets = [_zeros() for _ in range(k + 1)]
        for zs in zsets:
            jax.block_until_ready(zs)
        outs = sharded(*staged, *zsets[0])
        jax.block_until_ready(outs)          # warm
        t0 = time.perf_counter()
        outs = sharded(*staged, *zsets[1])
        jax.block_until_ready(outs)
        t1 = time.perf_counter()
        last = None
        for i in range(2, k + 1):
            last = sharded(*staged, *zsets[i])
        jax.block_until_ready(last)
        t2 = time.perf_counter()
        single = t1 - t0
        per = (t2 - t1) / (k - 1)
        return single, per

    return run, time_once, time_slope


def kernel(h, W1, b1, W2, b2, src, dst):
    h = np.asarray(h, np.float32)
    in_maps, meta = preprocess(h, np.asarray(W1, np.float32),
                               np.asarray(b1, np.float32),
                               np.asarray(W2, np.float32),
                               np.asarray(b2, np.float32), src, dst)
    nc = build_program(meta)
    run, _, _ = make_runner(nc, in_maps)
    results = run()
    out_new = np.concatenate([results[c]["out"] for c in range(NCORES)], 0)
    return out_new[meta["perm"]].astype(np.float32)

